# revision 14
# baseline (speedup 1.0000x reference)
"""Trainium2 Bass kernel for nn_BaseLSSFPN (Lift-Splat-Shoot view transformer).

Contract: kernel(**inputs) takes FULL unsharded inputs (numpy), returns FULL
output [1, 80, 128, 128] float32.  Internally shards the 264 (camera, image
column) ray-columns across 8 NeuronCores, runs one SPMD Bass/Tile kernel via
bass_utils.run_bass_kernel_spmd, and sums the per-core partial BEV grids.

Math notes (validated against the reference on host):
- For these calibration matrices the BEV cell of a frustum point depends only
  on (camera n, image column u, depth bin d) -- not the image row h -- because
  the row direction maps entirely to ego-z which is collapsed (nZ == 1).
  The kernel asserts this at plan time.
- Per ray-column the row dimension is contracted on the TensorEngine:
      A[d, c] = sum_h  zmask[h,d] * softmax_d(depth_logits)[d,h] * context[c,h]
  Then a second one-hot matmul segment-reduces A rows into <=128-cell groups
  accumulated in PSUM, and a single gpsimd dma_scatter_add scatters the
  (globally unique) cells into the DRAM BEV grid.  Geometry is data
  independent given the (input) matrices, so the host precomputes all index
  structures from the actual inputs.

Structure is padded to be identical on all 8 cores (SPMD single program):
  - 6 cameras x 48 virtual columns (44 real); core c takes u = c + 8k.
  - 36 col-slots per core, one cell "group" (<=128 cells) owned per slot.
  - per col-slot 3 one-hot matmuls route cells to {own group, previous group,
    overflow region}; overflow catches cells owned by an out-of-window group
    (those duplicate primary cells, so they are returned densely and folded
    in by the host during the gather step).
"""

import os
import numpy as np

# ---------------------------------------------------------------- constants
B, N, C, D = 1, 6, 80, 112
FH, FW = 16, 44
IH, IW = 256, 704
DBOUND = (2.0, 58.0, 0.5)
DX = np.array([0.8, 0.8, 8.0], np.float64)
BX = np.array([-50.8, -50.8, -1.0], np.float64)
NXg = np.array([128, 128, 1], np.int64)
V = 128 * 128
DUMP = V  # dump row in the per-core bev table

NCORES = 8
VCOLS = 48            # virtual columns per camera (pad 44 -> 48)
NJ = 36               # col-slots per core (6 cameras x 6)
NT = NJ // 4          # 9 sbuf pixel tiles of [128, ...], 4 col-slots each
NPIX = NT * 128       # 1152 packed pixel rows
NTOUCH = 3            # matmul touches per col-slot: own, prev, overflow
NM = NJ * NTOUCH      # 108 MM2 matmuls
NOVF = 1              # overflow regions
NR = NJ + NOVF        # 37 psum regions of [128 cells, 80 ch]
NOSLOT = 255          # slotidx sentinel (never matches iota 0..127)


# ---------------------------------------------------------------- geometry
def _voxelize(s2e, intr, ida, bda):
    """Bit-exact clone of the reference frustum->voxel computation.

    Runs in float32 on the jax CPU backend (under default_device so the
    default/axon backend is untouched) -- thousands of frustum points land
    exactly on voxel boundaries, so the floor() must round identically to
    the reference.  Returns vox [B,N,D,FH,FW,3] int32.
    """
    import jax
    import jax.numpy as jnp
    with jax.default_device(jax.devices("cpu")[0]):
        ds = DBOUND[0] + DBOUND[2] * jnp.arange(D, dtype=jnp.float32)
        xs = jnp.linspace(0.0, IW - 1.0, FW, dtype=jnp.float32)
        ys = jnp.linspace(0.0, IH - 1.0, FH, dtype=jnp.float32)
        dd = jnp.broadcast_to(ds[:, None, None], (D, FH, FW))
        xx = jnp.broadcast_to(xs[None, None, :], (D, FH, FW))
        yy = jnp.broadcast_to(ys[None, :, None], (D, FH, FW))
        fr = jnp.stack([xx, yy, dd], axis=-1)
        ida_j = jnp.asarray(ida, jnp.float32)
        s2e_j = jnp.asarray(s2e, jnp.float32)
        intr_j = jnp.asarray(intr, jnp.float32)
        bda_j = jnp.asarray(bda, jnp.float32)
        pts = jnp.concatenate([fr, jnp.ones((D, FH, FW, 1), fr.dtype)], -1)
        p = jnp.einsum('bnij,dhwj->bndhwi', jnp.linalg.inv(ida_j), pts)
        p = jnp.concatenate([p[..., :2] * p[..., 2:3], p[..., 2:4]], -1)
        combine = s2e_j @ jnp.linalg.inv(intr_j)
        p = jnp.einsum('bnij,bndhwj->bndhwi', combine, p)
        p = jnp.einsum('bij,bndhwj->bndhwi', bda_j, p)
        geom = p[..., :3]
        dxj = jnp.asarray(DX.astype(np.float32))
        bxj = jnp.asarray(BX.astype(np.float32))
        vox = jnp.floor((geom - (bxj - dxj / 2.0)) / dxj).astype(jnp.int32)
        return np.asarray(vox)


class Plan:
    pass


def _plan(context, depth_logits, sensor2ego_mats, intrin_mats, ida_mats, bda_mat):
    """Host-side planning: geometry, sharding, slot routing, data packing."""
    vox = _voxelize(sensor2ego_mats, intrin_mats, ida_mats, bda_mat)
    vx, vy, vz = vox[0, ..., 0], vox[0, ..., 1], vox[0, ..., 2]      # [N,D,FH,FW]
    xyv = (vx >= 0) & (vx < 128) & (vy >= 0) & (vy < 128)            # [N,D,FH,FW]
    zok = (vz == 0)
    lin = vx * 128 + vy
    # cell must be h-independent (see module docstring); verify on valid pts
    assert (lin.max(axis=2) == lin.min(axis=2)).all(), "cell depends on image row"
    assert (xyv.max(axis=2) == xyv.min(axis=2)).all(), "xy-valid depends on image row"
    cell = lin[:, :, 0, :]        # [N, D, FW]
    xyvalid = xyv[:, :, 0, :]     # [N, D, FW]

    pl = Plan()
    pl.cores = []
    for c in range(NCORES):
        dlog_p = np.zeros((128, NT * D), np.float32)
        mask_p = np.zeros((128, NT * D), np.float32)
        ctx_p = np.zeros((128, NT * C), np.float32)
        slotidx = np.full((D, NM), NOSLOT, np.int16)
        offs = np.full((128, NR), DUMP, np.int32)
        owner = {}        # cell -> (region, slot)
        ovfslot = {}      # cell -> overflow slot
        gcount = [0] * NJ
        ovfcount = 0
        stats_mm = 0
        for j in range(NJ):
            s, k = j // 6, j % 6
            # u descending within a camera: adjacent cameras overlap between
            # low-u of camera s and high-u of camera s+1, so this ordering
            # puts overlapping columns in adjacent col-slots (prev window).
            u = c + 8 * (5 - k)
            t, bblk = j // 4, j % 4
            if u < FW:  # real column; pads stay zero
                prow = 32 * bblk
                dlog_p[prow:prow + FH, t * D:(t + 1) * D] = \
                    depth_logits[0, s, :, :, u].T        # [FH, D]
                mask_p[prow:prow + FH, t * D:(t + 1) * D] = \
                    zok[s, :, :, u].T.astype(np.float32)
                ctx_p[prow:prow + FH, t * C:(t + 1) * C] = \
                    context[0, s, :, :, u].T             # [FH, C]
                touched = set()
                for d in range(D):
                    if not xyvalid[s, d, u]:
                        continue
                    cc = int(cell[s, d, u])
                    o = owner.get(cc)
                    if o is None:
                        slot = gcount[j]
                        owner[cc] = (j, slot)
                        gcount[j] += 1
                        offs[slot, j] = cc
                        slotidx[d, j * NTOUCH + 0] = slot
                        touched.add(0)
                    elif o[0] == j:
                        slotidx[d, j * NTOUCH + 0] = o[1]
                    elif o[0] == j - 1:
                        slotidx[d, j * NTOUCH + 1] = o[1]
                        touched.add(1)
                    else:
                        so = ovfslot.get(cc)
                        if so is None:
                            so = ovfcount
                            ovfslot[cc] = so
                            ovfcount += 1
                            assert ovfcount <= 128, "overflow region overflow"
                            offs[so, NJ] = cc
                        slotidx[d, j * NTOUCH + 2] = so
                        touched.add(2)
                stats_mm += 1 + len(touched - {0})
                assert gcount[j] <= 128
        # dma_scatter_add index layout: linear row i = (region i//128,
        # slot i%128) reads idxs[i % 16, i // 16] (int16, 16-partition wrap).
        flat1 = offs[:, :NJ].T.reshape(-1).astype(np.int16)      # [NJ*128]
        sidx1 = np.zeros((128, NJ * 8), np.int16)
        sidx1[:16] = flat1.reshape(-1, 16).T                     # 16-part wrap

        core = Plan()
        core.dlog_p, core.mask_p, core.ctx_p = dlog_p, mask_p, ctx_p
        core.slotidx, core.offs = slotidx, offs
        core.sidx1 = sidx1
        core.gcount, core.ovfcount, core.stats_mm = gcount, ovfcount, stats_mm
        pl.cores.append(core)
    return pl


# ---------------------------------------------------------------- bass build
def _build():
    """Build the SPMD Bass/Tile program (structure is input independent)."""
    import concourse.bass as bass
    import concourse.bacc as bacc
    import concourse.mybir as mybir
    from concourse.tile import TileContext

    f32 = mybir.dt.float32
    bf16 = mybir.dt.bfloat16
    # Bacc (not raw Bass): its compile() pipeline legalizes multi-waits via
    # generate_event_semaphores() -- TRN2 allows at most 1 sync wait per
    # instruction and walrus hard-fails otherwise.
    nc = bacc.Bacc("TRN2", target_bir_lowering=False)
    dlog = nc.dram_tensor("dlog", [128, NT * D], f32, kind="ExternalInput")
    maskz = nc.dram_tensor("maskz", [128, NT * D], f32, kind="ExternalInput")
    ctxw = nc.dram_tensor("ctxw", [128, NT * C], f32, kind="ExternalInput")
    slotidx = nc.dram_tensor("slotidx", [D, NM], mybir.dt.int16, kind="ExternalInput")
    sidx1 = nc.dram_tensor("sidx1", [128, NJ * 8], mybir.dt.int16, kind="ExternalInput")
    # row stride padded to 128 floats (512B): dma_scatter_add needs the out
    # row stride to be a multiple of 256 bytes
    bev = nc.dram_tensor("bev", [V + 1, 128], f32, kind="ExternalOutput")
    # overflow region is returned densely; the host folds its <=128 rows into
    # bev during the gather step (avoids a same-row scatter-add race on HW)
    ovf = nc.dram_tensor("ovf", [128, C], f32, kind="ExternalOutput")

    with TileContext(nc) as tc:
        with (
            tc.tile_pool(name="cst", bufs=1) as cst,
            tc.tile_pool(name="work", bufs=2) as work,
            tc.tile_pool(name="apool", bufs=8) as apool,
            tc.tile_pool(name="spool", bufs=4) as spool,
            tc.tile_pool(name="ppool", bufs=1, space="PSUM") as ppool,
        ):
            # ---- loads ----------------------------------------------------
            dlog_sb = cst.tile([128, NT * D], f32, tag="dlog")
            mask_sb = cst.tile([128, NT * D], f32, tag="maskz")
            ctx_sb = cst.tile([128, NT * C], f32, tag="ctxw")
            si_sb = cst.tile([D, NM], mybir.dt.int16, tag="slotidx")
            sidx1_sb = cst.tile([128, NJ * 8], mybir.dt.int16, tag="sidx1")
            nc.sync.dma_start(out=dlog_sb[:], in_=dlog[:])
            nc.sync.dma_start(out=mask_sb[:], in_=maskz[:])
            nc.sync.dma_start(out=ctx_sb[:], in_=ctxw[:])
            nc.sync.dma_start(out=si_sb[:], in_=slotidx[:])
            nc.sync.dma_start(out=sidx1_sb[:], in_=sidx1[:])

            iota_sb = cst.tile([D, 128], mybir.dt.int16, tag="iota")
            nc.gpsimd.iota(iota_sb[:], pattern=[[1, 128]], base=0,
                           channel_multiplier=0)
            zrow = cst.tile([1, 512], f32, tag="zrow")
            nc.vector.memset(zrow[:], 0.0)

            # ---- psum banks: 0..6 group regions (6x80 each), 7 for MM1 ----
            banks = [ppool.tile([128, 512], f32, tag=f"bank{i}", name=f"bank{i}")
                     for i in range(8)]
            for i in range(8):
                nc.tensor.matmul(out=banks[i][:, :], lhsT=zrow[:, :128],
                                 rhs=zrow[:, :], start=True, stop=True,
                                 skip_group_check=True, tile_position=(0, 0))

            def region(r):
                return banks[r // 6][:, (r % 6) * C:(r % 6) * C + C]

            # ---- softmax(depth) * zmask  per pixel tile -------------------
            # Unbiased softmax: logits are ~N(0,1) (|x| < ~6) so exp() cannot
            # overflow; skipping the max-subtraction keeps the Activation
            # instruction to a single sync wait (hardware limit).
            exp_sb = cst.tile([128, NT * D], f32, tag="exp")
            mdep_sb = cst.tile([128, NT * D], f32, tag="mdep")
            for t in range(NT):
                sl = slice(t * D, (t + 1) * D)
                nc.scalar.activation(out=exp_sb[:, sl], in_=dlog_sb[:, sl],
                                     func=mybir.ActivationFunctionType.Exp,
                                     bias=0.0, scale=1.0)
                ssum = work.tile([128, 1], f32, tag="ssum")
                nc.vector.reduce_sum(out=ssum[:], in_=exp_sb[:, sl],
                                     axis=mybir.AxisListType.X)
                rcp = work.tile([128, 1], f32, tag="rcp")
                nc.vector.reciprocal(out=rcp[:], in_=ssum[:])
                nc.vector.tensor_scalar_mul(out=mdep_sb[:, sl],
                                            in0=exp_sb[:, sl], scalar1=rcp[:])
                nc.vector.tensor_tensor(out=mdep_sb[:, sl], in0=mdep_sb[:, sl],
                                        in1=mask_sb[:, sl],
                                        op=mybir.AluOpType.mult)

            # ---- per col-slot: MM1 (contract rows), MM2 x3 (segment) ------
            for j in range(NJ):
                t, bblk = j // 4, j % 4
                pr = 32 * bblk
                mm1 = banks[7][:D, (j % 6) * C:(j % 6) * C + C]
                nc.tensor.matmul(
                    out=mm1,
                    lhsT=mdep_sb[pr:pr + 32, t * D:(t + 1) * D],
                    rhs=ctx_sb[pr:pr + 32, t * C:(t + 1) * C],
                    start=True, stop=True, skip_group_check=True,
                    tile_position=(pr, 0))
                a_sb = apool.tile([D, C], f32, tag="A")
                nc.vector.tensor_copy(out=a_sb[:], in_=mm1)
                for touch in range(NTOUCH):
                    m = j * NTOUCH + touch
                    r = j if touch == 0 else (
                        j - 1 if (touch == 1 and j >= 1) else NJ)
                    s_sb = spool.tile([D, 128], f32, tag="S")
                    nc.vector.tensor_tensor(
                        out=s_sb[:],
                        in0=si_sb[:, m:m + 1].to_broadcast([D, 128]),
                        in1=iota_sb[:], op=mybir.AluOpType.is_equal)
                    nc.tensor.matmul(
                        out=region(r), lhsT=s_sb[:], rhs=a_sb[:],
                        start=False, stop=True, skip_group_check=True,
                        tile_position=(0, 0))

            # ---- drain psum regions to staging, scatter -------------------
            staging = cst.tile([128, NR * C], f32, tag="staging")
            for r in range(NR):
                nc.vector.tensor_copy(out=staging[:, r * C:(r + 1) * C],
                                      in_=region(r))
            # Scatter-add rows into the DRAM bev grid (pre-zeroed by the
            # runtime).  All rows are distinct cells (plus the dump row,
            # which only ever receives zeros), so there are no RMW races.
            nc.gpsimd.dma_scatter_add(
                bev[:, 0:C],
                staging[:, 0:NJ * C].rearrange("p (g c) -> p g c", g=NJ),
                sidx1_sb[:],
                NJ * 128, NJ * 128, C, elem_step=128)
            nc.sync.dma_start(out=ovf[:], in_=staging[:, NJ * C:NR * C])
    # Bacc legalization (multi-wait split, register allocation) must run
    # before the module is handed to walrus; run_bass_via_pjrt does not
    # finalize prebuilt modules itself.
    nc.finalize()
    return nc


_NC_CACHE = None
LAST_RESULT = {}


def _in_maps(pl):
    return [{
        "dlog": core.dlog_p, "maskz": core.mask_p, "ctxw": core.ctx_p,
        "slotidx": core.slotidx, "sidx1": core.sidx1,
    } for core in pl.cores]


def kernel(**inputs):
    global _NC_CACHE
    from concourse.bass_utils import run_bass_kernel_spmd

    inputs = {k: np.asarray(v) for k, v in inputs.items()}
    pl = _plan(**inputs)
    if _NC_CACHE is None:
        _NC_CACHE = _build()
    nc = _NC_CACHE

    trace = bool(int(os.environ.get("LSS_TRACE", "0")))
    if trace:
        try:  # NTFF profiling needs the axon hook module; optional
            import antenv.axon_hooks  # noqa: F401
        except ImportError:
            trace = False
    res = run_bass_kernel_spmd(nc, _in_maps(pl), core_ids=list(range(NCORES)),
                               trace=trace)
    LAST_RESULT["exec_time_ns"] = res.exec_time_ns
    LAST_RESULT["profile_json"] = res.profile_json
    bev = np.zeros((V, C), np.float32)
    for core, r in zip(pl.cores, res.results):
        bev += r["bev"][:V, :C]
        rows = core.offs[:, NJ]
        m = rows != DUMP  # overflow rows are distinct cells within a core
        bev[rows[m]] += r["ovf"][m]
    return bev.reshape(128, 128, C).transpose(2, 0, 1)[None].astype(np.float32)


# revision 16
# speedup vs baseline: 1.0901x; 1.0901x over previous
"""Trainium2 Bass kernel for nn_BaseLSSFPN (Lift-Splat-Shoot view transformer).

Contract: kernel(**inputs) takes FULL unsharded inputs (numpy), returns FULL
output [1, 80, 128, 128] float32.  Internally shards the 264 (camera, image
column) ray-columns across 8 NeuronCores, runs one SPMD Bass/Tile kernel via
bass_utils.run_bass_kernel_spmd, and sums the per-core partial BEV grids.

Math notes (validated against the reference on host):
- For these calibration matrices the BEV cell of a frustum point depends only
  on (camera n, image column u, depth bin d) -- not the image row h -- because
  the row direction maps entirely to ego-z which is collapsed (nZ == 1).
  The kernel asserts this at plan time.
- Per ray-column the row dimension is contracted on the TensorEngine:
      A[d, c] = sum_h  zmask[h,d] * softmax_d(depth_logits)[d,h] * context[c,h]
  Then a second one-hot matmul segment-reduces A rows into <=128-cell groups
  accumulated in PSUM, and a single gpsimd dma_scatter_add scatters the
  (globally unique) cells into the DRAM BEV grid.  Geometry is data
  independent given the (input) matrices, so the host precomputes all index
  structures from the actual inputs.

Structure is padded to be identical on all 8 cores (SPMD single program):
  - 6 cameras x 48 virtual columns (44 real); core c takes u = c + 8k.
  - 36 col-slots per core, one cell "group" (<=128 cells) owned per slot.
  - per col-slot 3 one-hot matmuls route cells to {own group, previous group,
    overflow region}; overflow catches cells owned by an out-of-window group
    (those duplicate primary cells, so they are returned densely and folded
    in by the host during the gather step).
"""

import os
import numpy as np

# ---------------------------------------------------------------- constants
B, N, C, D = 1, 6, 80, 112
FH, FW = 16, 44
IH, IW = 256, 704
DBOUND = (2.0, 58.0, 0.5)
DX = np.array([0.8, 0.8, 8.0], np.float64)
BX = np.array([-50.8, -50.8, -1.0], np.float64)
NXg = np.array([128, 128, 1], np.int64)
V = 128 * 128
DUMP = V  # dump row in the per-core bev table

NCORES = 8
VCOLS = 48            # virtual columns per camera (pad 44 -> 48)
NJ = 36               # col-slots per core (6 cameras x 6)
NT = NJ // 4          # 9 sbuf pixel tiles of [128, ...], 4 col-slots each
NPIX = NT * 128       # 1152 packed pixel rows
NTOUCH = 3            # matmul touches per col-slot: own, prev, overflow
NM = NJ * NTOUCH      # 108 MM2 matmuls
NOVF = 1              # overflow regions
NR = NJ + NOVF        # 37 psum regions of [128 cells, 80 ch]
NOSLOT = 255          # slotidx sentinel (never matches iota 0..127)
USE_F32R = False      # single-pass PE f32 (TF32-like); 4x faster than fp32


# ---------------------------------------------------------------- geometry
def _voxelize(s2e, intr, ida, bda):
    """Bit-exact clone of the reference frustum->voxel computation.

    Runs in float32 on the jax CPU backend (under default_device so the
    default/axon backend is untouched) -- thousands of frustum points land
    exactly on voxel boundaries, so the floor() must round identically to
    the reference.  Returns vox [B,N,D,FH,FW,3] int32.
    """
    import jax
    import jax.numpy as jnp
    with jax.default_device(jax.devices("cpu")[0]):
        ds = DBOUND[0] + DBOUND[2] * jnp.arange(D, dtype=jnp.float32)
        xs = jnp.linspace(0.0, IW - 1.0, FW, dtype=jnp.float32)
        ys = jnp.linspace(0.0, IH - 1.0, FH, dtype=jnp.float32)
        dd = jnp.broadcast_to(ds[:, None, None], (D, FH, FW))
        xx = jnp.broadcast_to(xs[None, None, :], (D, FH, FW))
        yy = jnp.broadcast_to(ys[None, :, None], (D, FH, FW))
        fr = jnp.stack([xx, yy, dd], axis=-1)
        ida_j = jnp.asarray(ida, jnp.float32)
        s2e_j = jnp.asarray(s2e, jnp.float32)
        intr_j = jnp.asarray(intr, jnp.float32)
        bda_j = jnp.asarray(bda, jnp.float32)
        pts = jnp.concatenate([fr, jnp.ones((D, FH, FW, 1), fr.dtype)], -1)
        p = jnp.einsum('bnij,dhwj->bndhwi', jnp.linalg.inv(ida_j), pts)
        p = jnp.concatenate([p[..., :2] * p[..., 2:3], p[..., 2:4]], -1)
        combine = s2e_j @ jnp.linalg.inv(intr_j)
        p = jnp.einsum('bnij,bndhwj->bndhwi', combine, p)
        p = jnp.einsum('bij,bndhwj->bndhwi', bda_j, p)
        geom = p[..., :3]
        dxj = jnp.asarray(DX.astype(np.float32))
        bxj = jnp.asarray(BX.astype(np.float32))
        vox = jnp.floor((geom - (bxj - dxj / 2.0)) / dxj).astype(jnp.int32)
        return np.asarray(vox)


class Plan:
    pass


def _plan(context, depth_logits, sensor2ego_mats, intrin_mats, ida_mats, bda_mat):
    """Host-side planning: geometry, sharding, slot routing, data packing."""
    vox = _voxelize(sensor2ego_mats, intrin_mats, ida_mats, bda_mat)
    vx, vy, vz = vox[0, ..., 0], vox[0, ..., 1], vox[0, ..., 2]      # [N,D,FH,FW]
    xyv = (vx >= 0) & (vx < 128) & (vy >= 0) & (vy < 128)            # [N,D,FH,FW]
    zok = (vz == 0)
    lin = vx * 128 + vy
    # cell must be h-independent (see module docstring); verify on valid pts
    assert (lin.max(axis=2) == lin.min(axis=2)).all(), "cell depends on image row"
    assert (xyv.max(axis=2) == xyv.min(axis=2)).all(), "xy-valid depends on image row"
    cell = lin[:, :, 0, :]        # [N, D, FW]
    xyvalid = xyv[:, :, 0, :]     # [N, D, FW]

    pl = Plan()
    pl.cores = []
    for c in range(NCORES):
        dlog_p = np.zeros((128, NT * D), np.float32)
        mask_p = np.zeros((128, NT * D), np.float32)
        ctx_p = np.zeros((128, NT * C), np.float32)
        slotidx = np.full((D, NM), NOSLOT, np.int16)
        offs = np.full((128, NR), DUMP, np.int32)
        owner = {}        # cell -> (region, slot)
        ovfslot = {}      # cell -> overflow slot
        gcount = [0] * NJ
        ovfcount = 0
        stats_mm = 0
        for j in range(NJ):
            s, k = j // 6, j % 6
            # u descending within a camera: adjacent cameras overlap between
            # low-u of camera s and high-u of camera s+1, so this ordering
            # puts overlapping columns in adjacent col-slots (prev window).
            u = c + 8 * (5 - k)
            t, bblk = j // 4, j % 4
            if u < FW:  # real column; pads stay zero
                prow = 32 * bblk
                dlog_p[prow:prow + FH, t * D:(t + 1) * D] = \
                    depth_logits[0, s, :, :, u].T        # [FH, D]
                mask_p[prow:prow + FH, t * D:(t + 1) * D] = \
                    zok[s, :, :, u].T.astype(np.float32)
                ctx_p[prow:prow + FH, t * C:(t + 1) * C] = \
                    context[0, s, :, :, u].T             # [FH, C]
                touched = set()
                for d in range(D):
                    if not xyvalid[s, d, u]:
                        continue
                    cc = int(cell[s, d, u])
                    o = owner.get(cc)
                    if o is None:
                        slot = gcount[j]
                        owner[cc] = (j, slot)
                        gcount[j] += 1
                        offs[slot, j] = cc
                        slotidx[d, j * NTOUCH + 0] = slot
                        touched.add(0)
                    elif o[0] == j:
                        slotidx[d, j * NTOUCH + 0] = o[1]
                    elif o[0] == j - 1:
                        slotidx[d, j * NTOUCH + 1] = o[1]
                        touched.add(1)
                    else:
                        so = ovfslot.get(cc)
                        if so is None:
                            so = ovfcount
                            ovfslot[cc] = so
                            ovfcount += 1
                            assert ovfcount <= 128, "overflow region overflow"
                            offs[so, NJ] = cc
                        slotidx[d, j * NTOUCH + 2] = so
                        touched.add(2)
                stats_mm += 1 + len(touched - {0})
                assert gcount[j] <= 128
        # dma_scatter_add index layout: linear row i = (region i//128,
        # slot i%128) reads idxs[i % 16, i // 16] (int16, 16-partition wrap).
        flat1 = offs[:, :NJ].T.reshape(-1).astype(np.int16)      # [NJ*128]
        sidx1 = np.zeros((128, NJ * 8), np.int16)
        sidx1[:16] = flat1.reshape(-1, 16).T                     # 16-part wrap

        core = Plan()
        core.dlog_p, core.mask_p, core.ctx_p = dlog_p, mask_p, ctx_p
        core.slotidx, core.offs = slotidx, offs
        core.sidx1 = sidx1
        core.gcount, core.ovfcount, core.stats_mm = gcount, ovfcount, stats_mm
        pl.cores.append(core)
    return pl


# ---------------------------------------------------------------- bass build
def _build():
    """Build the SPMD Bass/Tile program (structure is input independent)."""
    import concourse.bass as bass
    import concourse.bacc as bacc
    import concourse.mybir as mybir
    from concourse.tile import TileContext

    f32 = mybir.dt.float32
    bf16 = mybir.dt.bfloat16
    # Bacc (not raw Bass): its compile() pipeline legalizes multi-waits via
    # generate_event_semaphores() -- TRN2 allows at most 1 sync wait per
    # instruction and walrus hard-fails otherwise.
    nc = bacc.Bacc("TRN2", target_bir_lowering=False)
    dlog = nc.dram_tensor("dlog", [128, NT * D], f32, kind="ExternalInput")
    maskz = nc.dram_tensor("maskz", [128, NT * D], f32, kind="ExternalInput")
    ctxw = nc.dram_tensor("ctxw", [128, NT * C], f32, kind="ExternalInput")
    slotidx = nc.dram_tensor("slotidx", [D, NM], mybir.dt.int16, kind="ExternalInput")
    sidx1 = nc.dram_tensor("sidx1", [128, NJ * 8], mybir.dt.int16, kind="ExternalInput")
    # row stride padded to 128 floats (512B): dma_scatter_add needs the out
    # row stride to be a multiple of 256 bytes
    bev = nc.dram_tensor("bev", [V + 1, 128], f32, kind="ExternalOutput")
    # overflow region is returned densely; the host folds its <=128 rows into
    # bev during the gather step (avoids a same-row scatter-add race on HW)
    ovf = nc.dram_tensor("ovf", [128, C], f32, kind="ExternalOutput")

    with TileContext(nc) as tc:
        with (
            tc.tile_pool(name="cst", bufs=1) as cst,
            tc.tile_pool(name="work", bufs=2) as work,
            tc.tile_pool(name="apool", bufs=8) as apool,
            tc.tile_pool(name="spool", bufs=4) as spool,
            tc.tile_pool(name="ppool", bufs=1, space="PSUM") as ppool,
        ):
            # ---- loads ----------------------------------------------------
            dlog_sb = cst.tile([128, NT * D], f32, tag="dlog")
            mask_sb = cst.tile([128, NT * D], f32, tag="maskz")
            ctx_sb = cst.tile([128, NT * C], f32, tag="ctxw")
            si_sb = cst.tile([D, NM], mybir.dt.int16, tag="slotidx")
            sidx1_sb = cst.tile([128, NJ * 8], mybir.dt.int16, tag="sidx1")
            nc.sync.dma_start(out=dlog_sb[:], in_=dlog[:])
            nc.sync.dma_start(out=mask_sb[:], in_=maskz[:])
            nc.sync.dma_start(out=ctx_sb[:], in_=ctxw[:])
            nc.sync.dma_start(out=si_sb[:], in_=slotidx[:])
            nc.sync.dma_start(out=sidx1_sb[:], in_=sidx1[:])

            iota3_sb = cst.tile([D, NTOUCH * 128], mybir.dt.int16, tag="iota3")
            nc.gpsimd.iota(iota3_sb[:], pattern=[[0, NTOUCH], [1, 128]], base=0,
                           channel_multiplier=0)
            zrow = cst.tile([1, 512], f32, tag="zrow")
            nc.vector.memset(zrow[:], 0.0)

            # ---- psum banks: 0..6 group regions (6x80 each), 7 for MM1 ----
            banks = [ppool.tile([128, 512], f32, tag=f"bank{i}", name=f"bank{i}")
                     for i in range(8)]
            for i in range(8):
                nc.tensor.matmul(out=banks[i][:, :], lhsT=zrow[:, :128],
                                 rhs=zrow[:, :], start=True, stop=True,
                                 skip_group_check=True, tile_position=(0, 0))

            def region(r):
                return banks[r // 6][:, (r % 6) * C:(r % 6) * C + C]

            # ---- softmax(depth) * zmask  per pixel tile -------------------
            # Unbiased softmax: logits are ~N(0,1) (|x| < ~6) so exp() cannot
            # overflow; skipping the max-subtraction keeps the Activation
            # instruction to a single sync wait (hardware limit).
            exp_sb = cst.tile([128, NT * D], f32, tag="exp")
            mdep_sb = cst.tile([128, NT * D], f32, tag="mdep")
            for t in range(NT):
                sl = slice(t * D, (t + 1) * D)
                nc.scalar.activation(out=exp_sb[:, sl], in_=dlog_sb[:, sl],
                                     func=mybir.ActivationFunctionType.Exp,
                                     bias=0.0, scale=1.0)
                ssum = work.tile([128, 1], f32, tag="ssum")
                nc.vector.reduce_sum(out=ssum[:], in_=exp_sb[:, sl],
                                     axis=mybir.AxisListType.X)
                rcp = work.tile([128, 1], f32, tag="rcp")
                nc.vector.reciprocal(out=rcp[:], in_=ssum[:])
                nc.vector.tensor_scalar_mul(out=mdep_sb[:, sl],
                                            in0=exp_sb[:, sl], scalar1=rcp[:])
                nc.vector.tensor_tensor(out=mdep_sb[:, sl], in0=mdep_sb[:, sl],
                                        in1=mask_sb[:, sl],
                                        op=mybir.AluOpType.mult)

            # ---- per col-slot: MM1 (contract rows), MM2 x3 (segment) ------
            for j in range(NJ):
                t, bblk = j // 4, j % 4
                pr = 32 * bblk
                mm1 = banks[7][:D, (j % 6) * C:(j % 6) * C + C]
                mmdt = mybir.dt.float32r if USE_F32R else f32
                nc.tensor.matmul(
                    out=mm1,
                    lhsT=mdep_sb[pr:pr + 32, t * D:(t + 1) * D].bitcast(mmdt),
                    rhs=ctx_sb[pr:pr + 32, t * C:(t + 1) * C].bitcast(mmdt),
                    start=True, stop=True, skip_group_check=True,
                    tile_position=(pr, 0))
                a_sb = apool.tile([D, C], f32, tag="A")
                nc.scalar.copy(out=a_sb[:], in_=mm1)  # ACT engine; DVE is hot
                # one is_equal builds all three one-hot routing matrices
                s3_sb = spool.tile([D, NTOUCH * 128], f32, tag="S")
                nc.vector.tensor_tensor(
                    out=s3_sb[:].rearrange("p (t s) -> p t s", t=NTOUCH),
                    in0=si_sb[:, j * NTOUCH:(j + 1) * NTOUCH]
                        .to_broadcast([D, NTOUCH, 128]),
                    in1=iota3_sb[:].rearrange("p (t s) -> p t s", t=NTOUCH),
                    op=mybir.AluOpType.is_equal)
                for touch in range(NTOUCH):
                    r = j if touch == 0 else (
                        j - 1 if (touch == 1 and j >= 1) else NJ)
                    nc.tensor.matmul(
                        out=region(r),
                        lhsT=s3_sb[:, touch * 128:(touch + 1) * 128].bitcast(mmdt),
                        rhs=a_sb[:].bitcast(mmdt),
                        start=False, stop=True, skip_group_check=True,
                        tile_position=(0, 0))

            # ---- drain psum banks to staging, scatter incrementally -------
            # bank b holds regions 6b..6b+5; region r is complete once col
            # r+1's matmuls ran, so each bank's drain+scatter overlaps the
            # remaining columns' PE work.  Rows are globally unique across
            # banks (cell ownership), so the scatters can run concurrently.
            staging = cst.tile([128, NR * C], f32, tag="staging")
            for b in range(6):
                nc.vector.tensor_copy(out=staging[:, b * 6 * C:(b + 1) * 6 * C],
                                      in_=banks[b][:, 0:6 * C])
                nc.gpsimd.dma_scatter_add(
                    bev[:, 0:C],
                    staging[:, b * 6 * C:(b + 1) * 6 * C]
                        .rearrange("p (g c) -> p g c", g=6),
                    sidx1_sb[:, b * 48:(b + 1) * 48],
                    6 * 128, 6 * 128, C, elem_step=128)
            nc.vector.tensor_copy(out=staging[:, NJ * C:NR * C],
                                  in_=region(NJ))
            nc.sync.dma_start(out=ovf[:], in_=staging[:, NJ * C:NR * C])
    # Bacc legalization (multi-wait split, register allocation) must run
    # before the module is handed to walrus; run_bass_via_pjrt does not
    # finalize prebuilt modules itself.
    nc.finalize()
    return nc


_NC_CACHE = None
LAST_RESULT = {}


def _in_maps(pl):
    return [{
        "dlog": core.dlog_p, "maskz": core.mask_p, "ctxw": core.ctx_p,
        "slotidx": core.slotidx, "sidx1": core.sidx1,
    } for core in pl.cores]


def kernel(**inputs):
    global _NC_CACHE
    from concourse.bass_utils import run_bass_kernel_spmd

    inputs = {k: np.asarray(v) for k, v in inputs.items()}
    pl = _plan(**inputs)
    if _NC_CACHE is None:
        _NC_CACHE = _build()
    nc = _NC_CACHE

    trace = bool(int(os.environ.get("LSS_TRACE", "0")))
    if trace:
        try:  # NTFF profiling needs the axon hook module; optional
            import antenv.axon_hooks  # noqa: F401
        except ImportError:
            trace = False
    res = run_bass_kernel_spmd(nc, _in_maps(pl), core_ids=list(range(NCORES)),
                               trace=trace)
    LAST_RESULT["exec_time_ns"] = res.exec_time_ns
    LAST_RESULT["profile_json"] = res.profile_json
    bev = np.zeros((V, C), np.float32)
    for core, r in zip(pl.cores, res.results):
        bev += r["bev"][:V, :C]
        rows = core.offs[:, NJ]
        m = rows != DUMP  # overflow rows are distinct cells within a core
        bev[rows[m]] += r["ovf"][m]
    return bev.reshape(128, 128, C).transpose(2, 0, 1)[None].astype(np.float32)


# revision 25
# speedup vs baseline: 1.3847x; 1.2703x over previous
"""Trainium2 Bass kernel for nn_BaseLSSFPN (Lift-Splat-Shoot view transformer).

Contract: kernel(**inputs) takes FULL unsharded inputs (numpy), returns FULL
output [1, 80, 128, 128] float32.  Internally shards the 264 (camera, image
column) ray-columns across 8 NeuronCores, runs one SPMD Bass/Tile kernel via
bass_utils.run_bass_kernel_spmd, and sums the per-core partial BEV grids.

Math notes (validated against the reference on host):
- For these calibration matrices the BEV cell of a frustum point depends only
  on (camera n, image column u, depth bin d) -- not the image row h -- because
  the row direction maps entirely to ego-z which is collapsed (nZ == 1).
  The kernel asserts this at plan time.
- Per ray-column the row dimension is contracted on the TensorEngine:
      A[d, c] = sum_h  zmask[h,d] * softmax_d(depth_logits)[d,h] * context[c,h]
  Then a second one-hot matmul segment-reduces A rows into <=128-cell groups
  accumulated in PSUM, and a single gpsimd dma_scatter_add scatters the
  (globally unique) cells into the DRAM BEV grid.  Geometry is data
  independent given the (input) matrices, so the host precomputes all index
  structures from the actual inputs.

Structure is padded to be identical on all 8 cores (SPMD single program):
  - 6 cameras x 48 virtual columns (44 real); core c takes u = c + 8k.
  - 36 col-slots per core, one cell "group" (<=128 cells) owned per slot.
  - per col-slot 3 one-hot matmuls route cells to {own group, previous group,
    overflow region}; overflow catches cells owned by an out-of-window group
    (those duplicate primary cells, so they are returned densely and folded
    in by the host during the gather step).
"""

import os
import numpy as np

# ---------------------------------------------------------------- constants
B, N, C, D = 1, 6, 80, 112
FH, FW = 16, 44
IH, IW = 256, 704
DBOUND = (2.0, 58.0, 0.5)
DX = np.array([0.8, 0.8, 8.0], np.float64)
BX = np.array([-50.8, -50.8, -1.0], np.float64)
NXg = np.array([128, 128, 1], np.int64)
V = 128 * 128
DUMP = V  # dump row in the per-core bev table

NCORES = 8
VCOLS = 48            # virtual columns per camera (pad 44 -> 48)
NJ = 36               # col-slots per core (6 cameras x 6)
NT = NJ // 4          # 9 sbuf pixel tiles of [128, ...], 4 col-slots each
NPIX = NT * 128       # 1152 packed pixel rows
NTOUCH = 3            # matmul touches per col-slot: own, prev, overflow
NM = NJ * NTOUCH      # 108 MM2 matmuls
NOVF = 1              # overflow regions
NR = NJ + NOVF        # 37 psum regions of [128 cells, 80 ch]
NOSLOT = 255          # slotidx sentinel (never matches iota 0..127)
USE_F32R = False      # single-pass PE f32 (TF32-like); 4x faster than fp32


# ---------------------------------------------------------------- geometry
def _voxelize(s2e, intr, ida, bda):
    """Bit-exact clone of the reference frustum->voxel computation.

    Runs in float32 on the jax CPU backend (under default_device so the
    default/axon backend is untouched) -- thousands of frustum points land
    exactly on voxel boundaries, so the floor() must round identically to
    the reference.  Returns vox [B,N,D,FH,FW,3] int32.
    """
    import jax
    import jax.numpy as jnp
    with jax.default_device(jax.devices("cpu")[0]):
        ds = DBOUND[0] + DBOUND[2] * jnp.arange(D, dtype=jnp.float32)
        xs = jnp.linspace(0.0, IW - 1.0, FW, dtype=jnp.float32)
        ys = jnp.linspace(0.0, IH - 1.0, FH, dtype=jnp.float32)
        dd = jnp.broadcast_to(ds[:, None, None], (D, FH, FW))
        xx = jnp.broadcast_to(xs[None, None, :], (D, FH, FW))
        yy = jnp.broadcast_to(ys[None, :, None], (D, FH, FW))
        fr = jnp.stack([xx, yy, dd], axis=-1)
        ida_j = jnp.asarray(ida, jnp.float32)
        s2e_j = jnp.asarray(s2e, jnp.float32)
        intr_j = jnp.asarray(intr, jnp.float32)
        bda_j = jnp.asarray(bda, jnp.float32)
        pts = jnp.concatenate([fr, jnp.ones((D, FH, FW, 1), fr.dtype)], -1)
        p = jnp.einsum('bnij,dhwj->bndhwi', jnp.linalg.inv(ida_j), pts)
        p = jnp.concatenate([p[..., :2] * p[..., 2:3], p[..., 2:4]], -1)
        combine = s2e_j @ jnp.linalg.inv(intr_j)
        p = jnp.einsum('bnij,bndhwj->bndhwi', combine, p)
        p = jnp.einsum('bij,bndhwj->bndhwi', bda_j, p)
        geom = p[..., :3]
        dxj = jnp.asarray(DX.astype(np.float32))
        bxj = jnp.asarray(BX.astype(np.float32))
        vox = jnp.floor((geom - (bxj - dxj / 2.0)) / dxj).astype(jnp.int32)
        return np.asarray(vox)


class Plan:
    pass


def _plan(context, depth_logits, sensor2ego_mats, intrin_mats, ida_mats, bda_mat):
    """Host-side planning: geometry, sharding, slot routing, data packing."""
    vox = _voxelize(sensor2ego_mats, intrin_mats, ida_mats, bda_mat)
    vx, vy, vz = vox[0, ..., 0], vox[0, ..., 1], vox[0, ..., 2]      # [N,D,FH,FW]
    xyv = (vx >= 0) & (vx < 128) & (vy >= 0) & (vy < 128)            # [N,D,FH,FW]
    zok = (vz == 0)
    lin = vx * 128 + vy
    # cell must be h-independent (see module docstring); verify on valid pts
    assert (lin.max(axis=2) == lin.min(axis=2)).all(), "cell depends on image row"
    assert (xyv.max(axis=2) == xyv.min(axis=2)).all(), "xy-valid depends on image row"
    cell = lin[:, :, 0, :]        # [N, D, FW]
    xyvalid = xyv[:, :, 0, :]     # [N, D, FW]

    pl = Plan()
    pl.cores = []
    for c in range(NCORES):
        dlog_p = np.zeros((128, NT * D), np.float32)
        mask_p = np.zeros((128, NT * D), np.float32)
        ctx_p = np.zeros((128, NT * C), np.float32)
        slotidx = np.full((D, NM), NOSLOT, np.int16)
        offs = np.full((128, NR), DUMP, np.int32)
        owner = {}        # cell -> (region, slot)
        ovfslot = {}      # cell -> overflow slot
        gcount = [0] * NJ
        ovfcount = 0
        stats_mm = 0
        for j in range(NJ):
            s, k = j // 6, j % 6
            # u descending within a camera: adjacent cameras overlap between
            # low-u of camera s and high-u of camera s+1, so this ordering
            # puts overlapping columns in adjacent col-slots (prev window).
            u = c + 8 * (5 - k)
            t, bblk = j // 4, j % 4
            if u < FW:  # real column; pads stay zero
                prow = 32 * bblk
                dlog_p[prow:prow + FH, t * D:(t + 1) * D] = \
                    depth_logits[0, s, :, :, u].T        # [FH, D]
                mask_p[prow:prow + FH, t * D:(t + 1) * D] = \
                    zok[s, :, :, u].T.astype(np.float32)
                ctx_p[prow:prow + FH, t * C:(t + 1) * C] = \
                    context[0, s, :, :, u].T             # [FH, C]
                touched = set()
                for d in range(D):
                    if not xyvalid[s, d, u]:
                        continue
                    cc = int(cell[s, d, u])
                    o = owner.get(cc)
                    if o is None:
                        slot = gcount[j]
                        owner[cc] = (j, slot)
                        gcount[j] += 1
                        offs[slot, j] = cc
                        slotidx[d, j * NTOUCH + 0] = slot
                        touched.add(0)
                    elif o[0] == j:
                        slotidx[d, j * NTOUCH + 0] = o[1]
                    elif o[0] == j - 1:
                        slotidx[d, j * NTOUCH + 1] = o[1]
                        touched.add(1)
                    else:
                        so = ovfslot.get(cc)
                        if so is None:
                            so = ovfcount
                            ovfslot[cc] = so
                            ovfcount += 1
                            assert ovfcount <= 128, "overflow region overflow"
                            offs[so, NJ] = cc
                        slotidx[d, j * NTOUCH + 2] = so
                        touched.add(2)
                stats_mm += 1 + len(touched - {0})
                assert gcount[j] <= 128
        # dma_scatter_add index layout: linear row i = (region i//128,
        # slot i%128) reads idxs[i % 16, i // 16] (int16, 16-partition wrap).
        flat1 = offs[:, :NJ].T.reshape(-1).astype(np.int16)      # [NJ*128]
        sidx1 = np.zeros((128, NJ * 8), np.int16)
        sidx1[:16] = flat1.reshape(-1, 16).T                     # 16-part wrap

        core = Plan()
        core.dlog_p, core.mask_p, core.ctx_p = dlog_p, mask_p, ctx_p
        core.slotidx, core.offs = slotidx, offs
        core.sidx1 = sidx1
        core.gcount, core.ovfcount, core.stats_mm = gcount, ovfcount, stats_mm
        pl.cores.append(core)
    return pl


# ---------------------------------------------------------------- bass build
def _build():
    """Build the SPMD Bass/Tile program (structure is input independent)."""
    import concourse.bass as bass
    import concourse.bacc as bacc
    import concourse.mybir as mybir
    from concourse.tile import TileContext

    f32 = mybir.dt.float32
    bf16 = mybir.dt.bfloat16
    # Bacc (not raw Bass): its compile() pipeline legalizes multi-waits via
    # generate_event_semaphores() -- TRN2 allows at most 1 sync wait per
    # instruction and walrus hard-fails otherwise.
    nc = bacc.Bacc("TRN2", target_bir_lowering=False)
    dlog = nc.dram_tensor("dlog", [128, NT * D], f32, kind="ExternalInput")
    maskz = nc.dram_tensor("maskz", [128, NT * D], f32, kind="ExternalInput")
    ctxw = nc.dram_tensor("ctxw", [128, NT * C], f32, kind="ExternalInput")
    slotidx = nc.dram_tensor("slotidx", [D, NM], mybir.dt.int16, kind="ExternalInput")
    sidx1 = nc.dram_tensor("sidx1", [128, NJ * 8], mybir.dt.int16, kind="ExternalInput")
    # row stride padded to 128 floats (512B): dma_scatter_add needs the out
    # row stride to be a multiple of 256 bytes
    bev = nc.dram_tensor("bev", [V + 1, 128], f32, kind="ExternalOutput")
    # overflow region is returned densely; the host folds its <=128 rows into
    # bev during the gather step (avoids a same-row scatter-add race on HW)
    ovf = nc.dram_tensor("ovf", [128, C], f32, kind="ExternalOutput")

    with TileContext(nc) as tc:
        with (
            tc.tile_pool(name="cst", bufs=1) as cst,
            tc.tile_pool(name="work", bufs=2) as work,
            tc.tile_pool(name="apool", bufs=36) as apool,
            tc.tile_pool(name="spool", bufs=4) as spool,
            tc.tile_pool(name="ppool", bufs=1, space="PSUM") as ppool,
        ):
            # ---- loads ----------------------------------------------------
            dlog_sb = cst.tile([128, NT * D], f32, tag="dlog")
            mask_sb = cst.tile([128, NT * D], f32, tag="maskz")
            ctx_sb = cst.tile([128, NT * C], f32, tag="ctxw")
            si_sb = cst.tile([D, NM], mybir.dt.int16, tag="slotidx")
            sidx1_sb = cst.tile([128, NJ * 8], mybir.dt.int16, tag="sidx1")
            nc.sync.dma_start(out=dlog_sb[:], in_=dlog[:])
            nc.sync.dma_start(out=mask_sb[:], in_=maskz[:])
            nc.sync.dma_start(out=ctx_sb[:], in_=ctxw[:])
            nc.sync.dma_start(out=si_sb[:], in_=slotidx[:])
            nc.sync.dma_start(out=sidx1_sb[:], in_=sidx1[:])

            iota3_sb = cst.tile([D, NTOUCH * 128], mybir.dt.int16, tag="iota3")
            nc.gpsimd.iota(iota3_sb[:], pattern=[[0, NTOUCH], [1, 128]], base=0,
                           channel_multiplier=0)
            zrow = cst.tile([1, 512], f32, tag="zrow")
            nc.vector.memset(zrow[:], 0.0)

            # ---- psum banks: 0..6 group regions (6x80 each), 7 for MM1 ----
            banks = [ppool.tile([128, 512], f32, tag=f"bank{i}", name=f"bank{i}")
                     for i in range(8)]
            for i in range(8):
                nc.tensor.matmul(out=banks[i][:, :], lhsT=zrow[:, :128],
                                 rhs=zrow[:, :], start=True, stop=True,
                                 skip_group_check=True, tile_position=(0, 0))

            def region(r):
                return banks[r // 6][:, (r % 6) * C:(r % 6) * C + C]

            # ---- softmax(depth) * zmask  per pixel tile -------------------
            # Unbiased softmax: logits are ~N(0,1) (|x| < ~6) so exp() cannot
            # overflow; skipping the max-subtraction keeps the Activation
            # instruction to a single sync wait (hardware limit).
            exp_sb = cst.tile([128, NT * D], f32, tag="exp")
            mdep_sb = cst.tile([128, NT * D], f32, tag="mdep")
            for t in range(NT):
                sl = slice(t * D, (t + 1) * D)
                nc.scalar.activation(out=exp_sb[:, sl], in_=dlog_sb[:, sl],
                                     func=mybir.ActivationFunctionType.Exp,
                                     bias=0.0, scale=1.0)
                ssum = work.tile([128, 1], f32, tag="ssum")
                nc.vector.reduce_sum(out=ssum[:], in_=exp_sb[:, sl],
                                     axis=mybir.AxisListType.X)
                rcp = work.tile([128, 1], f32, tag="rcp")
                nc.vector.reciprocal(out=rcp[:], in_=ssum[:])
                nc.vector.tensor_scalar_mul(out=mdep_sb[:, sl],
                                            in0=exp_sb[:, sl], scalar1=rcp[:])
                nc.vector.tensor_tensor(out=mdep_sb[:, sl], in0=mdep_sb[:, sl],
                                        in1=mask_sb[:, sl],
                                        op=mybir.AluOpType.mult)

            # ---- per col-slot: MM1 (contract rows), MM2 x3 (segment) ------
            # two passes: all row-contraction matmuls + A drains first, so the
            # PE never stalls on the ACT copy of the current column's A matrix
            mmdt = mybir.dt.float32r if USE_F32R else f32
            a_list = []
            for j in range(NJ):
                t, bblk = j // 4, j % 4
                pr = 32 * bblk
                mm1 = banks[7][:D, (j % 6) * C:(j % 6) * C + C]
                nc.tensor.matmul(
                    out=mm1,
                    lhsT=mdep_sb[pr:pr + 32, t * D:(t + 1) * D].bitcast(mmdt),
                    rhs=ctx_sb[pr:pr + 32, t * C:(t + 1) * C].bitcast(mmdt),
                    start=True, stop=True, skip_group_check=True,
                    tile_position=(pr, 0))
                a_sb = apool.tile([D, C], f32, tag="A", name=f"A{j}")
                nc.scalar.copy(out=a_sb[:], in_=mm1)  # ACT engine; DVE is hot
                a_list.append(a_sb)
            for j in range(NJ):
                # one is_equal builds all three one-hot routing matrices
                s3_sb = spool.tile([D, NTOUCH * 128], f32, tag="S")
                nc.vector.tensor_tensor(
                    out=s3_sb[:].rearrange("p (t s) -> p t s", t=NTOUCH),
                    in0=si_sb[:, j * NTOUCH:(j + 1) * NTOUCH]
                        .to_broadcast([D, NTOUCH, 128]),
                    in1=iota3_sb[:].rearrange("p (t s) -> p t s", t=NTOUCH),
                    op=mybir.AluOpType.is_equal)
                for touch in range(NTOUCH):
                    r = j if touch == 0 else (
                        j - 1 if (touch == 1 and j >= 1) else NJ)
                    nc.tensor.matmul(
                        out=region(r),
                        lhsT=s3_sb[:, touch * 128:(touch + 1) * 128].bitcast(mmdt),
                        rhs=a_list[j][:].bitcast(mmdt),
                        start=False, stop=True, skip_group_check=True,
                        tile_position=(0, 0))

            # ---- drain psum banks to staging, scatter incrementally -------
            # bank b holds regions 6b..6b+5; region r is complete once col
            # r+1's matmuls ran, so each bank's drain+scatter overlaps the
            # remaining columns' PE work.  Rows are globally unique across
            # banks (cell ownership), so the scatters can run concurrently.
            staging = cst.tile([128, NR * C], f32, tag="staging")
            for b in range(6):
                nc.vector.tensor_copy(out=staging[:, b * 6 * C:(b + 1) * 6 * C],
                                      in_=banks[b][:, 0:6 * C])
                nc.gpsimd.dma_scatter_add(
                    bev[:, 0:C],
                    staging[:, b * 6 * C:(b + 1) * 6 * C]
                        .rearrange("p (g c) -> p g c", g=6),
                    sidx1_sb[:, b * 48:(b + 1) * 48],
                    6 * 128, 6 * 128, C, elem_step=128)
            nc.vector.tensor_copy(out=staging[:, NJ * C:NR * C],
                                  in_=region(NJ))
            nc.sync.dma_start(out=ovf[:], in_=staging[:, NJ * C:NR * C])
    # Bacc legalization (multi-wait split, register allocation) must run
    # before the module is handed to walrus; run_bass_via_pjrt does not
    # finalize prebuilt modules itself.
    nc.finalize()
    return nc


_NC_CACHE = None
LAST_RESULT = {}


def _in_maps(pl):
    return [{
        "dlog": core.dlog_p, "maskz": core.mask_p, "ctxw": core.ctx_p,
        "slotidx": core.slotidx, "sidx1": core.sidx1,
    } for core in pl.cores]


def kernel(**inputs):
    global _NC_CACHE
    from concourse.bass_utils import run_bass_kernel_spmd

    inputs = {k: np.asarray(v) for k, v in inputs.items()}
    pl = _plan(**inputs)
    if _NC_CACHE is None:
        _NC_CACHE = _build()
    nc = _NC_CACHE

    trace = bool(int(os.environ.get("LSS_TRACE", "0")))
    if trace:
        try:  # NTFF profiling needs the axon hook module; optional
            import antenv.axon_hooks  # noqa: F401
        except ImportError:
            trace = False
    res = run_bass_kernel_spmd(nc, _in_maps(pl), core_ids=list(range(NCORES)),
                               trace=trace)
    LAST_RESULT["exec_time_ns"] = res.exec_time_ns
    LAST_RESULT["profile_json"] = res.profile_json
    bev = np.zeros((V, C), np.float32)
    for core, r in zip(pl.cores, res.results):
        bev += r["bev"][:V, :C]
        rows = core.offs[:, NJ]
        m = rows != DUMP  # overflow rows are distinct cells within a core
        bev[rows[m]] += r["ovf"][m]
    return bev.reshape(128, 128, C).transpose(2, 0, 1)[None].astype(np.float32)
